# revision 16
# baseline (speedup 1.0000x reference)
"""AttentionPooling kernel for TRN2 (8 NeuronCores, data-parallel over batch).

Reference computation (per batch b, per span n):
  x = token_reps + sinusoidal_pe                     (S, H)
  window = [start_n, end_n)  (width <= 32, all indices in-range)
  q shared across spans; per-head scores over the window -> softmax -> pool V
  attn_out = ctx @ Wo^T + out_b; y1 = LN(attn_out + dq)
  y = LN(relu(y1@W1^T+b1)@W2^T+b2 + y1); zero masked spans

Key reformulation (no gather): the query is shared, so per-token scores
  ts[s,h] = x[s] . (scale * Wk_h^T q_h)
are computed once per batch.  Softmax over a span's window of a shared
length-S score vector + pooling of per-token values v_tok = x @ Wv^T
becomes two masked matmuls with the 0/1 window matrix M[n,s]:
  ctx[n] = (M @ (e * v_tok)) / (M @ e),   e[s,h] = exp(ts[s,h])
(score bias c_h cancels in softmax; bv folds into out_b via sum attn = 1).

Matmul operands are bf16 (fp32 PSUM accumulation): fp32/fp32r matmuls run
at 2-4 cyc/row on the PE while bf16 runs at 1 cyc/row with fast weight
load, and bf16 halves weight DMA traffic.  Measured end-to-end error vs
the f32 reference is ~4e-3 relative.

Each core handles one batch element: B=8 == n_cores.
"""

import numpy as np
import ml_dtypes

import concourse.bass as bass
import concourse.bacc as bacc
import concourse.mybir as mybir
import concourse.tile as tile
from concourse.bass_utils import run_bass_kernel_spmd
from concourse.masks import make_identity

B, S, N, H = 8, 512, 512, 768
NH, HD = 4, 192
F = 3072
NT = S // 128   # 4  partition tiles over s or n
KH = H // 128   # 6  partition tiles over hidden dim
KF = F // 128   # 24 partition tiles over ffn dim
f32 = mybir.dt.float32
bf16 = mybir.dt.bfloat16
AF = mybir.ActivationFunctionType
OP = mybir.AluOpType
BF = ml_dtypes.bfloat16


def _mm(nc, out, lhsT, rhs, start, stop):
    nc.tensor.matmul(out, lhsT, rhs, start=start, stop=stop)


def _bcast_row(nc, dst, handle, n):
    # DMA-broadcast a length-n DRAM vector across 128 partitions.
    nc.sync.dma_start(out=dst, in_=bass.AP(handle, 0, [[0, 128], [1, n]]))


def _layer_norm_core(nc, pool, out_ap, in_ap, eps_t):
    # y = (x - mu) * rsqrt(var + eps) over free dim (768) of (128, 768).
    stats = pool.tile([128, 3, 6], f32, tag="ln_stats")
    for c in range(3):
        nc.vector.bn_stats(out=stats[:, c, :], in_=in_ap[:, c * 256 : (c + 1) * 256])
    mv = pool.tile([128, 2], f32, tag="ln_mv")
    nc.vector.bn_aggr(out=mv, in_=stats)
    std = pool.tile([128, 1], f32, tag="ln_std")
    nc.scalar.activation(
        out=std, in_=mv[:, 1:2], func=AF.Sqrt, bias=eps_t[:, 0:1], scale=1.0
    )
    rstd = pool.tile([128, 1], f32, tag="ln_rstd")
    nc.vector.reciprocal(out=rstd, in_=std)
    nc.vector.tensor_scalar(
        out=out_ap,
        in0=in_ap,
        scalar1=mv[:, 0:1],
        scalar2=rstd[:, 0:1],
        op0=OP.subtract,
        op1=OP.mult,
    )


def build_bass(apply_gb=False, apply_b2=False, apply_b1=False, apply_mask=False):
    from contextlib import ExitStack

    nc = bacc.Bacc("TRN2", target_bir_lowering=False, debug=False)

    x_d = nc.dram_tensor("x", [S, H], bf16, kind="ExternalInput")
    wvus_d = nc.dram_tensor("wvus", [H, H + 4], bf16, kind="ExternalInput")
    wot_d = nc.dram_tensor("wot", [H, H], bf16, kind="ExternalInput")
    w1t_d = nc.dram_tensor("w1t", [H, F], bf16, kind="ExternalInput")
    w2t_d = nc.dram_tensor("w2t", [F, H], bf16, kind="ExternalInput")
    starts_d = nc.dram_tensor("starts", [N], f32, kind="ExternalInput")
    ends_d = nc.dram_tensor("ends", [N], f32, kind="ExternalInput")
    addv_d = nc.dram_tensor("addv", [H], f32, kind="ExternalInput")
    out_d = nc.dram_tensor("out", [N, H], f32, kind="ExternalOutput")
    if apply_mask:
        maskc_d = nc.dram_tensor("maskc", [128, NT], f32, kind="ExternalInput")
    if apply_b1:
        b1c_d = nc.dram_tensor("b1c", [128, KF], f32, kind="ExternalInput")
    if apply_b2:
        b2_d = nc.dram_tensor("b2", [H], f32, kind="ExternalInput")
    if apply_gb:
        lng_d = nc.dram_tensor("lng", [H], f32, kind="ExternalInput")
        lnb_d = nc.dram_tensor("lnb", [H], f32, kind="ExternalInput")

    out_ap = out_d.ap()

    with tile.TileContext(nc) as tc:
        with (
            tc.tile_pool(name="singles", bufs=1) as singles,
            tc.tile_pool(name="y1p", bufs=1) as y1p,
            tc.tile_pool(name="w1p", bufs=1) as w1p,
            tc.tile_pool(name="w2p", bufs=1) as w2p,
            tc.tile_pool(name="ffp", bufs=1) as ffp,
            tc.tile_pool(name="outp", bufs=2) as outp,
            tc.tile_pool(name="lnp", bufs=2) as lnp,
        ):
            # --- identities + iota first so the PE can start ASAP ---
            ident_bf = singles.tile([128, 128], bf16)
            make_identity(nc, ident_bf)
            ident_f32 = singles.tile([128, 128], f32)
            make_identity(nc, ident_f32)
            iota_c = singles.tile([128, NT], f32)
            nc.gpsimd.iota(
                iota_c,
                pattern=[[128, NT]],
                base=0,
                channel_multiplier=1,
                allow_small_or_imprecise_dtypes=True,
            )
            eps_t = singles.tile([128, 1], f32)
            nc.vector.memset(eps_t, 1e-5)
            zero_t = singles.tile([128, 1], f32)
            nc.vector.memset(zero_t, 0.0)

            # HAM warm-up: ~4us of dummy transposes while input DMAs land,
            # so stages A/B run at 2.4GHz instead of the cold 1.2GHz.
            with tc.tile_pool(name="psW", bufs=1, space="PSUM") as psW:
                wps = psW.tile([128, S], bf16, tag="wps")
                for r in range(10):
                    for c in range(NT):
                        nc.tensor.transpose(
                            wps[:, c * 128 : (c + 1) * 128], ident_bf, ident_bf
                        )

            # --- big input DMAs (sync queue) in priority order ---
            xa = singles.tile([128, NT, H], bf16)
            x_r = x_d.ap().rearrange("(t p) h -> p t h", p=128)
            for st in range(NT):
                nc.sync.dma_start(out=xa[:, st, :], in_=x_r[:, st, :])
            wv = singles.tile([128, KH, H + 4], bf16)
            nc.sync.dma_start(
                out=wv, in_=wvus_d.ap().rearrange("(t p) h -> p t h", p=128)
            )

            # --- small broadcasts (gpsimd queue) ---
            starts_b = singles.tile([128, S], f32)
            _bcast_row(nc, starts_b, starts_d, S)
            ends_b = singles.tile([128, S], f32)
            _bcast_row(nc, ends_b, ends_d, S)
            addv_b = singles.tile([128, H], f32)
            _bcast_row(nc, addv_b, addv_d, H)
            maskc_t = b1c_t = b2_b = g_b = b_b = None
            if apply_mask:
                maskc_t = singles.tile([128, NT], f32)
                nc.sync.dma_start(out=maskc_t, in_=maskc_d.ap())
            if apply_b1:
                b1c_t = singles.tile([128, KF], f32)
                nc.sync.dma_start(out=b1c_t, in_=b1c_d.ap())
            if apply_b2:
                b2_b = singles.tile([128, H], f32)
                _bcast_row(nc, b2_b, b2_d, H)
            if apply_gb:
                g_b = singles.tile([128, H], f32)
                _bcast_row(nc, g_b, lng_d, H)
                b_b = singles.tile([128, H], f32)
                _bcast_row(nc, b_b, lnb_d, H)

            def ln_full(out_ap_, in_ap_):
                _layer_norm_core(nc, lnp, out_ap_, in_ap_, eps_t)
                if apply_gb:
                    nc.vector.tensor_mul(out=out_ap_, in0=out_ap_, in1=g_b)
                    nc.vector.tensor_add(out=out_ap_, in0=out_ap_, in1=b_b)

            # --- window masks mT[s, n] on the (otherwise idle) gpsimd ---
            mT = []
            for st in range(NT):
                m_t = singles.tile([128, S], bf16, tag=f"mT{st}", name=f"mT{st}")
                tmp = singles.tile([128, S], f32, tag=f"mtmp{st}", name=f"mtmp{st}")
                nc.vector.tensor_scalar(
                    out=tmp, in0=starts_b,
                    scalar1=iota_c[:, st : st + 1], scalar2=None, op0=OP.is_le,
                )
                nc.vector.tensor_scalar(
                    out=m_t, in0=ends_b,
                    scalar1=iota_c[:, st : st + 1], scalar2=None, op0=OP.is_gt,
                )
                nc.vector.tensor_mul(out=m_t, in0=m_t.bitcast(bf16), in1=tmp)
                mT.append(m_t)

            y1 = [y1p.tile([128, H], f32, tag=f"y1_{i}", name=f"y1_{i}")
                  for i in range(NT)]

            es_ctx = ExitStack()
            ctxp = es_ctx.enter_context(tc.tile_pool(name="ctxp", bufs=1))
            ctx_t = [ctxp.tile([128, H], bf16, tag=f"ctx{i}", name=f"ctx{i}")
                     for i in range(NT)]

            es1 = ExitStack()
            xTp = es1.enter_context(tc.tile_pool(name="xTp", bufs=1))
            evp = es1.enter_context(tc.tile_pool(name="evp", bufs=1))

            # --- stage A: transpose x to xT (bf16) ---
            xT = []
            es_psB = ExitStack()
            psB = es_psB.enter_context(tc.tile_pool(name="psB", bufs=3, space="PSUM"))
            es_psA = ExitStack()
            psA = es_psA.enter_context(tc.tile_pool(name="psA", bufs=2, space="PSUM"))
            if True:
              for jt in range(KH):
                ps = psA.tile([128, S], bf16, tag="psA", name=f"psA{jt}")
                for st in range(NT):
                    nc.tensor.transpose(
                        ps[:, st * 128 : (st + 1) * 128],
                        xa[:, st, jt * 128 : (jt + 1) * 128],
                        ident_bf,
                    )
                xt = xTp.tile([128, S], bf16, tag=f"xT{jt}")
                nc.vector.tensor_copy(out=xt, in_=ps)
                xT.append(xt)

            # --- stage B: v_tok | ts = x @ [WvT | Us]; e = exp(ts); ev ---
            ev = []
            if True:
              for st in range(NT):
                psv = psB.tile([128, H + 4], f32, tag="psv", name=f"psB{st}")
                for kt in range(KH):
                    lhsT = xT[kt][:, st * 128 : (st + 1) * 128]
                    _mm(nc, psv[:, 0:512], lhsT, wv[:, kt, 0:512],
                        kt == 0, kt == KH - 1)
                    _mm(nc, psv[:, 512:772], lhsT, wv[:, kt, 512:772],
                        kt == 0, kt == KH - 1)
                e_f = lnp.tile([128, 4], f32, tag="e_f", bufs=4)
                nc.scalar.activation(out=e_f, in_=psv[:, 768:772], func=AF.Exp, bias=zero_t[:, 0:1])
                evt = evp.tile([128, H + 4], bf16, tag=f"ev{st}")
                nc.scalar.copy(out=evt[:, 768:772], in_=e_f)
                for h in range(NH):
                    nc.vector.tensor_scalar_mul(
                        out=evt[:, h * HD : (h + 1) * HD],
                        in0=psv[:, h * HD : (h + 1) * HD],
                        scalar1=e_f[:, h : h + 1],
                    )
                ev.append(evt)

            es_psA.close()

            # big FFN weights: issue now so they stream behind x/wv
            w1r = w1p.tile([128, KH, F], bf16)
            nc.sync.dma_start(
                out=w1r, in_=w1t_d.ap().rearrange("(t p) h -> p t h", p=128)
            )
            w2r = w2p.tile([128, KF, H], bf16)
            nc.sync.dma_start(
                out=w2r, in_=w2t_d.ap().rearrange("(t p) h -> p t h", p=128)
            )

            # --- stage D: ctx_unnorm = M @ [ev | e]; normalize via ACT ---
            if True:
              for nt in range(NT):
                psc = psB.tile([128, H + 4], f32, tag="psv", name=f"psD{nt}")
                for st in range(NT):
                    lhsT = mT[st][:, nt * 128 : (nt + 1) * 128]
                    _mm(nc, psc[:, 0:512], lhsT, ev[st][:, 0:512],
                        st == 0, st == NT - 1)
                    _mm(nc, psc[:, 512:772], lhsT, ev[st][:, 512:772],
                        st == 0, st == NT - 1)
                rz = lnp.tile([128, 4], f32, tag="rz")
                nc.vector.tensor_scalar_max(
                    out=rz, in0=psc[:, 768:772], scalar1=1e-30
                )
                nc.vector.reciprocal(out=rz, in_=rz)
                for h in range(2):
                    nc.scalar.activation(
                        out=ctx_t[nt][:, h * HD : (h + 1) * HD],
                        in_=psc[:, h * HD : (h + 1) * HD],
                        func=AF.Copy,
                        scale=rz[:, h : h + 1],
                    )
                for h in range(2, NH):
                    nc.vector.tensor_scalar_mul(
                        out=ctx_t[nt][:, h * HD : (h + 1) * HD],
                        in0=psc[:, h * HD : (h + 1) * HD],
                        scalar1=rz[:, h : h + 1],
                    )

            es_psB.close()
            es1.close()  # free xT/ev

            with (
                tc.tile_pool(name="ctxTp", bufs=1) as ctxTp,
                tc.tile_pool(name="wotp", bufs=1) as wotp,
                tc.tile_pool(name="y1Tp", bufs=1) as y1Tp,
            ):
                wotr = wotp.tile([128, KH, H], bf16)
                nc.sync.dma_start(
                    out=wotr, in_=wot_d.ap().rearrange("(t p) h -> p t h", p=128)
                )

                es_psF = ExitStack()
                psF = es_psF.enter_context(
                    tc.tile_pool(name="psF", bufs=2, space="PSUM"))
                es_psE = ExitStack()
                psE = es_psE.enter_context(
                    tc.tile_pool(name="psE", bufs=2, space="PSUM"))

                # --- stage E: transpose ctx -> ctxT ---
                ctxT = []
                for jt in range(KH):
                    ps = psE.tile([128, S], bf16, tag="psE", name=f"psE{jt}")
                    for st in range(NT):
                        nc.tensor.transpose(
                            ps[:, st * 128 : (st + 1) * 128],
                            ctx_t[st][:, jt * 128 : (jt + 1) * 128],
                            ident_bf,
                        )
                    ct = ctxTp.tile([128, S], bf16, tag=f"ctxT{jt}")
                    nc.scalar.copy(out=ct, in_=ps)
                    ctxT.append(ct)

                # --- stage F: attn_out = ctx@WoT (+addv) ; LN1 -> y1 ---
                for nt in range(NT):
                    psa = psF.tile([128, H + 4], f32, tag="psa", name=f"psF{nt}")
                    for kt in range(KH):
                        lhsT = ctxT[kt][:, nt * 128 : (nt + 1) * 128]
                        _mm(nc, psa[:, 0:512], lhsT, wotr[:, kt, 0:512],
                            kt == 0, kt == KH - 1)
                        _mm(nc, psa[:, 512:768], lhsT, wotr[:, kt, 512:768],
                            kt == 0, kt == KH - 1)
                    ao = lnp.tile([128, H], f32, tag="ao")
                    nc.vector.tensor_add(out=ao, in0=psa[:, 0:768], in1=addv_b)
                    ln_full(y1[nt][:, :], ao[:, :])

                es_psE.close()
                es_psG = ExitStack()
                psG = es_psG.enter_context(
                    tc.tile_pool(name="psG", bufs=2, space="PSUM"))

                # --- stage G: y1 -> y1T (bf16), overlaps F via shared region ---
                y1T = []
                for jt in range(KH):
                    ps = psG.tile([128, S], f32, tag="psG", name=f"psG{jt}")
                    for st in range(NT):
                        nc.tensor.transpose(
                            ps[:, st * 128 : (st + 1) * 128],
                            y1[st][:, jt * 128 : (jt + 1) * 128],
                            ident_f32,
                        )
                    yt = y1Tp.tile([128, S], bf16, tag=f"y1T{jt}")
                    nc.scalar.copy(out=yt, in_=ps)
                    y1T.append(yt)

                es_psG.close()
                es_psF.close()

                es_psHI = ExitStack()
                psI = es_psHI.enter_context(
                    tc.tile_pool(name="psI", bufs=2, space="PSUM"))
                psH = es_psHI.enter_context(
                    tc.tile_pool(name="psH", bufs=3, space="PSUM"))

                # --- stage H: ff1 = relu(W1 @ y1T + b1) ---
                ff1 = []
                for mt in range(KF):
                    psf = psH.tile([128, S], f32, tag="psf", name=f"psH{mt}")
                    for kt in range(KH):
                        _mm(nc, psf, w1r[:, kt, mt * 128 : (mt + 1) * 128],
                            y1T[kt], kt == 0, kt == KH - 1)
                    fft = ffp.tile([128, S], bf16, tag=f"ff{mt}")
                    nc.scalar.activation(
                        out=fft, in_=psf, func=AF.Relu,
                        bias=(b1c_t[:, mt : mt + 1] if apply_b1
                              else zero_t[:, 0:1]),
                        scale=1.0,
                    )
                    ff1.append(fft)

                # --- stage I: y2pre = ff1@W2T, mt-outer so evac overlaps ---
                for mt in range(NT):
                    psy = psI.tile([128, H + 4], f32, tag="psy", name=f"psI{mt}")
                    for kt in range(KF):
                        lhsT = ff1[kt][:, mt * 128 : (mt + 1) * 128]
                        _mm(nc, psy[:, 0:512], lhsT, w2r[:, kt, 0:512],
                            kt == 0, kt == KF - 1)
                        _mm(nc, psy[:, 512:768], lhsT, w2r[:, kt, 512:768],
                            kt == 0, kt == KF - 1)
                    y2 = outp.tile([128, H], f32, tag="y2")
                    nc.vector.tensor_add(out=y2, in0=psy[:, 0:768], in1=y1[mt])
                    if apply_b2:
                        nc.vector.tensor_add(out=y2, in0=y2, in1=b2_b)
                    yf = outp.tile([128, H], f32, tag="yf")
                    ln_full(yf[:, :], y2[:, :])
                    if apply_mask:
                        nc.vector.tensor_scalar_mul(
                            out=yf, in0=yf, scalar1=maskc_t[:, mt : mt + 1]
                        )
                    nc.sync.dma_start(
                        out=out_ap[mt * 128 : (mt + 1) * 128, :], in_=yf
                    )
                es_psHI.close()

            es_ctx.close()

    nc.compile()
    return nc


def _sinusoidal_pe():
    pos = np.arange(S, dtype=np.float32)[:, None]
    div = np.exp(
        np.arange(0, H, 2, dtype=np.float32) * (-np.log(10000.0) / H)
    ).astype(np.float32)
    ang = pos * div  # (S, H/2)
    pe = np.stack([np.sin(ang), np.cos(ang)], axis=-1).reshape(S, H)
    return pe.astype(np.float32)


def make_host_data(inputs):
    """Host-side constant folding. Returns (shared, per_core, flags)."""
    tok = np.asarray(inputs["token_reps"], dtype=np.float32)
    ids = np.asarray(inputs["span_ids"])
    msk = np.asarray(inputs["span_masks"]).astype(np.float32)
    dq = np.asarray(inputs["dummy_query"], dtype=np.float32)[0, 0]
    ipw = np.asarray(inputs["in_proj_w"], dtype=np.float32)
    ipb = np.asarray(inputs["in_proj_b"], dtype=np.float32)
    out_w = np.asarray(inputs["out_w"], dtype=np.float32)
    out_b = np.asarray(inputs["out_b"], dtype=np.float32)
    lng = np.asarray(inputs["ln_g"], dtype=np.float32)
    lnb = np.asarray(inputs["ln_b"], dtype=np.float32)
    w1 = np.asarray(inputs["ffn_w1"], dtype=np.float32)
    b1 = np.asarray(inputs["ffn_b1"], dtype=np.float32)
    w2 = np.asarray(inputs["ffn_w2"], dtype=np.float32)
    b2 = np.asarray(inputs["ffn_b2"], dtype=np.float32)

    wq, wk, wv = ipw[:H], ipw[H : 2 * H], ipw[2 * H :]
    bq, bk, bv = ipb[:H], ipb[H : 2 * H], ipb[2 * H :]

    q = (dq @ wq.T + bq).astype(np.float32)  # (H,)
    scale = np.float32(1.0 / np.sqrt(HD))
    # Us[:, h] = scale * Wk_h^T q_h  (the constant q.bk_h cancels in softmax)
    Us = np.zeros((H, NH), dtype=np.float32)
    for h in range(NH):
        qh = q[h * HD : (h + 1) * HD]
        Us[:, h] = scale * (wk[h * HD : (h + 1) * HD, :].T @ qh)

    flags = {
        "apply_gb": not (np.all(lng == 1.0) and np.all(lnb == 0.0)),
        "apply_b2": bool(np.any(b2 != 0.0)),
        "apply_b1": bool(np.any(b1 != 0.0)),
        "apply_mask": not np.all(msk == 1.0),
    }

    shared = {
        "wvus": np.ascontiguousarray(
            np.concatenate([wv.T, Us], axis=1).astype(BF)
        ),
        "wot": np.ascontiguousarray(out_w.T.astype(BF)),
        "w1t": np.ascontiguousarray(w1.T.astype(BF)),
        "w2t": np.ascontiguousarray(w2.T.astype(BF)),
        # residual is the RAW dummy query dq, not the projected q
        "addv": np.ascontiguousarray(out_b + out_w @ bv + dq, dtype=np.float32),
    }
    if flags["apply_b1"]:
        shared["b1c"] = np.ascontiguousarray(b1.reshape(KF, 128).T, np.float32)
    if flags["apply_b2"]:
        shared["b2"] = np.ascontiguousarray(b2, dtype=np.float32)
    if flags["apply_gb"]:
        shared["lng"] = np.ascontiguousarray(lng, dtype=np.float32)
        shared["lnb"] = np.ascontiguousarray(lnb, dtype=np.float32)

    pe = _sinusoidal_pe()
    per_core = []
    for b in range(B):
        starts = ids[b, :, 0].astype(np.float32)
        widths = (ids[b, :, 1] - ids[b, :, 0]).astype(np.float32)
        ends = starts + widths * msk[b]
        pc = {
            "x": np.ascontiguousarray((tok[b] + pe).astype(BF)),
            "starts": np.ascontiguousarray(starts),
            "ends": np.ascontiguousarray(ends),
        }
        if flags["apply_mask"]:
            pc["maskc"] = np.ascontiguousarray(
                msk[b].reshape(NT, 128).T, dtype=np.float32
            )
        per_core.append(pc)
    return shared, per_core, flags


_NC_CACHE = {}


def kernel(**inputs) -> np.ndarray:
    shared, per_core, flags = make_host_data(inputs)
    in_maps = [{**shared, **pc} for pc in per_core]
    key = tuple(sorted(flags.items()))
    if key not in _NC_CACHE:
        _NC_CACHE[key] = build_bass(**flags)
    res = run_bass_kernel_spmd(_NC_CACHE[key], in_maps, core_ids=list(range(B)))
    return np.stack([r["out"] for r in res.results], axis=0)
